# revision 10
# baseline (speedup 1.0000x reference)
"""KANLinear forward on 8 TRN2 NeuronCores.

Reference computes
    out = x @ base_w.T + base_b + spline_w @ linspace(0, 1, S)
The spline branch is batch-independent, so it folds into a single bias
vector on the host. The device kernel is a data-parallel matmul: each
core computes a [2048, 1024] batch shard as out.T tiles ([out-feature
partitions, batch free dim]) so the per-feature bias is a per-partition
scalar add fused into the PSUM->SBUF eviction.

Matmuls run in float16 (PE streams 16-bit operands at 1 row/cycle at
2.4 GHz vs fp32r's effective half rate) with fp32 PSUM accumulation;
x/w are rounded to fp16 on the host and the output is stored fp16
(rel err ~3e-4, under the 2e-3 gate). Inputs are pre-tiled on the host
into the exact SBUF layouts so every DMA line is a contiguous >=2KB
per-partition run:
  x  -> [NB, 128, KO, 512]   (nb b-tile, ki partition, ko k-subtile, b col)
  w  -> [128, MO, KO, 128]   (ki partition, mo o-tile, ko k-subtile, m col)
  out <- [NB, 128, MO, 512]  (nb, o-partition, mo o-tile, b col)

Schedule (traced): ~7us fixed runtime preamble precedes the first
user DMA; HWDGE rings stream queued transfers back-to-back (receipt
only delays semaphore visibility by ~2us). So: w rides the ACT ring
(singles first, then pairs — PE eats one 256KB w tile per 1.7us), x
rides the SP ring (nb0 in k-halves for an early PE start, then whole
1MB nb stripes), outputs ride SWDGE + whichever HWDGE ring is idle.
While the first chunks are in flight the tensor engine runs ~44 dummy
128-wide matmuls on a zeroed tile so the PE_HAM clock gate is already
at 8/8 (2.4 GHz) when real data lands — otherwise the first ~3.4us of
matmuls run at 1.2 GHz.
"""

import numpy as np

import concourse.bass as bass  # noqa: F401
import concourse.mybir as mybir
import concourse.tile as tile
from concourse import bacc
from concourse.bass_utils import run_bass_kernel_spmd

B, IN, OUT = 16384, 1024, 1024
N_CORES = 8
BS = B // N_CORES  # 2048 batch rows per core
P = 128  # SBUF partitions
KO = IN // P  # 8 k-subtiles of the contraction dim
MO = OUT // P  # 8 out-feature tiles (psum partition dim)
NB_TILE = 512  # matmul free dim = one fp32 PSUM bank
NB = BS // NB_TILE  # 4 batch tiles per core
N_WARM = 60  # dummy matmuls to hold PE_HAM at 8/8 until real data lands

_CACHE = {}


def _build_nc():
    f32 = mybir.dt.float32
    f16 = mybir.dt.float16

    nc = bacc.Bacc("TRN2", target_bir_lowering=False)
    x_d = nc.dram_tensor("x_t", [NB, P, KO, NB_TILE], f16, kind="ExternalInput")
    w_d = nc.dram_tensor("w_t", [P, MO, KO, P], f16, kind="ExternalInput")
    b_d = nc.dram_tensor("bias_t", [P, MO], f32, kind="ExternalInput")
    o_d = nc.dram_tensor("out_t", [NB, P, MO, NB_TILE], f16, kind="ExternalOutput")

    with tile.TileContext(nc) as tc:
        with (
            tc.tile_pool(name="wp", bufs=1) as wp,
            tc.tile_pool(name="xp", bufs=1) as xp,
            tc.tile_pool(name="cp", bufs=1) as cp,
            tc.tile_pool(name="op", bufs=1) as op,
            tc.tile_pool(name="ps", bufs=4, space="PSUM") as ps,
            tc.tile_pool(name="pw", bufs=1, space="PSUM") as pw,
        ):
            # PE warmup: zero tile -> dummy matmuls keep the PE busy (and
            # the HAM clock-gate warming) while the first DMAs are in
            # flight. Results land in a scratch PSUM bank, never read.
            wz = cp.tile([P, P], f16)
            nc.vector.memset(wz[:], 0.0)
            psz = pw.tile([P, P], f32)
            for _ in range(N_WARM):
                nc.tensor.matmul(psz[:], wz[:], wz[:], start=True, stop=True)

            # bias rides SWDGE (idle until outputs start)
            bias_sb = cp.tile([P, MO], f32)
            nc.gpsimd.dma_start(bias_sb[:], b_d[:])

            w_sb = [None] * MO
            x_parts = [[] for _ in range(NB)]

            def load_w(mos, engine):
                t = wp.tile([P, len(mos), KO, P], f16, tag=f"w{mos[0]}")
                engine.dma_start(t[:], w_d[:, mos[0] : mos[0] + len(mos)])
                for i, mo in enumerate(mos):
                    w_sb[mo] = t[:, i]

            def load_x(nb, k0, kn, engine):
                t = xp.tile([P, kn, NB_TILE], f16, tag=f"x{nb}_{k0}")
                engine.dma_start(t[:], x_d[nb, :, k0 : k0 + kn])
                x_parts[nb].append((k0, kn, t))

            # ALL inputs ride one ring (SP HWDGE) in exact PE consumption
            # order. Two concurrent input queues share the 16 SDMA engines
            # round-robin at *packet* granularity, so the queue with bigger
            # per-partition runs starves the other (traced: w got 115GB/s
            # vs x 218GB/s -> 3.5us mid-kernel stalls). One queue at ~350GB/s
            # delivers every tile ahead of its first use instead.
            KH = KO // 2
            load_w([0], nc.sync)
            load_x(0, 0, KH, nc.sync)
            load_x(0, KH, KH, nc.sync)
            load_w([1], nc.sync)
            load_w([2], nc.sync)
            load_w([3], nc.sync)
            load_w([4], nc.sync)
            load_x(1, 0, KO, nc.sync)
            load_w([5], nc.sync)
            load_w([6], nc.sync)
            load_w([7], nc.sync)
            load_x(2, 0, KO, nc.sync)
            load_x(3, 0, KO, nc.sync)

            def x_slice(nb, k):
                for k0, kn, t in x_parts[nb]:
                    if k0 <= k < k0 + kn:
                        return t[:, k - k0]
                raise AssertionError

            MH = MO // 2  # output DMA chunk = half an nb stripe (512KB)
            out_engines = {
                (0, 0): nc.gpsimd,
                (0, 1): nc.scalar,
                (1, 0): nc.gpsimd,
                (1, 1): nc.scalar,
                (2, 0): nc.gpsimd,
                (2, 1): nc.scalar,
            }
            # last nb ships per-mo (128KB chunks) so the tail after the
            # final matmul is one small transfer; the final chunk rides
            # the SP ring (fastest first-byte, input work long done).
            nb3_eng = [nc.gpsimd, nc.scalar, nc.gpsimd, nc.scalar,
                       nc.gpsimd, nc.scalar, nc.gpsimd, nc.sync]

            for nb in range(NB):
                if nb < NB - 1:
                    ot = [op.tile([P, MH, NB_TILE], f16, tag=f"o{nb}_{h}",
                                  name=f"o{nb}_{h}")
                          for h in range(2)]
                else:
                    ot = [op.tile([P, NB_TILE], f16, tag=f"o{nb}_{q}",
                                  name=f"o{nb}_{q}")
                          for q in range(MO)]
                for mo in range(MO):
                    pt = ps.tile([P, NB_TILE], mybir.dt.float32)
                    for k in range(KO):
                        nc.tensor.matmul(
                            pt[:],
                            w_sb[mo][:, k],
                            x_slice(nb, k),
                            start=(k == 0),
                            stop=(k == KO - 1),
                        )
                    if nb < NB - 1:
                        h, i = divmod(mo, MH)
                        dst = ot[h][:, i]
                        nc.vector.tensor_scalar_add(
                            dst, pt[:], bias_sb[:, mo : mo + 1]
                        )
                        if mo == MH - 1:
                            out_engines[(nb, 0)].dma_start(
                                o_d[nb, :, 0:MH], ot[0][:]
                            )
                        elif mo == MO - 1:
                            out_engines[(nb, 1)].dma_start(
                                o_d[nb, :, MH:MO], ot[1][:]
                            )
                    elif mo < MO - 1:
                        nc.vector.tensor_scalar_add(
                            ot[mo][:], pt[:], bias_sb[:, mo : mo + 1]
                        )
                        nb3_eng[mo].dma_start(o_d[nb, :, mo], ot[mo][:])
                    else:
                        # final tile: evict in halves on two engines in
                        # parallel, each issuing its own half-DMA, so the
                        # tail after the last matmul is minimal.
                        NH = NB_TILE // 2
                        nc.vector.tensor_scalar_add(
                            ot[mo][:, :NH], pt[:, :NH], bias_sb[:, mo : mo + 1]
                        )
                        nc.sync.dma_start(
                            o_d[nb, :, mo, :NH], ot[mo][:, :NH]
                        )
                        nc.vector.tensor_scalar_add(
                            ot[mo][:, NH:], pt[:, NH:], bias_sb[:, mo : mo + 1]
                        )
                        nc.scalar.dma_start(
                            o_d[nb, :, mo, NH:], ot[mo][:, NH:]
                        )

    nc.finalize()
    return nc


def _get_nc():
    if "nc" not in _CACHE:
        _CACHE["nc"] = _build_nc()
    return _CACHE["nc"]


def _prep_inputs(x, base_w, base_b, spline_w):
    x = np.asarray(x, dtype=np.float32)
    base_w = np.asarray(base_w, dtype=np.float32)
    base_b = np.asarray(base_b, dtype=np.float32)
    spline_w = np.asarray(spline_w, dtype=np.float32)

    s_feats = spline_w.shape[1]
    spline_input = np.linspace(0.0, 1.0, s_feats, dtype=np.float32)
    bias = (base_b + spline_w @ spline_input).astype(np.float32)  # [OUT]

    # w_dev[ki, mo, ko, m] = base_w[mo*P + m, ko*P + ki]
    w_dev = np.ascontiguousarray(
        base_w.astype(np.float16).reshape(MO, P, KO, P).transpose(3, 0, 2, 1)
    )
    # bias_dev[p, mo] = bias[mo*P + p]
    bias_dev = np.ascontiguousarray(bias.reshape(MO, P).T)

    x16 = x.astype(np.float16)
    in_maps = []
    for c in range(N_CORES):
        xs = x16[c * BS : (c + 1) * BS]  # [BS, IN]
        # x_dev[nb, ki, ko, col] = xs[nb*NB_TILE + col, ko*P + ki]
        x_dev = np.ascontiguousarray(
            xs.reshape(NB, NB_TILE, KO, P).transpose(0, 3, 2, 1)
        )
        in_maps.append({"x_t": x_dev, "w_t": w_dev, "bias_t": bias_dev})
    return in_maps


def _run(inputs, trace=False, tmpdir=None):
    nc = _get_nc()
    in_maps = _prep_inputs(**inputs)
    res = run_bass_kernel_spmd(
        nc, in_maps, core_ids=list(range(N_CORES)), trace=trace, tmpdir=tmpdir
    )
    outs = []
    for c in range(N_CORES):
        arr = np.asarray(res.results[c]["out_t"])  # [NB, P, MO, NB_TILE] f16
        # out_core[nb*NB_TILE + col, mo*P + p] = arr[nb, p, mo, col]
        outs.append(arr.transpose(0, 3, 2, 1).reshape(BS, OUT))
    full = np.concatenate(outs, axis=0).astype(np.float32)
    return np.ascontiguousarray(full), res


def kernel(**inputs) -> np.ndarray:
    out, _ = _run(inputs, trace=False)
    return out


# revision 11
# speedup vs baseline: 1.1685x; 1.1685x over previous
"""KANLinear forward on 8 TRN2 NeuronCores.

Reference computes
    out = x @ base_w.T + base_b + spline_w @ linspace(0, 1, S)
The spline branch is batch-independent, so it folds into a single bias
vector on the host. The device kernel is a data-parallel matmul: each
core computes a [2048, 1024] batch shard as out.T tiles ([out-feature
partitions, batch free dim]) so the per-feature bias is a per-partition
scalar add fused into the PSUM->SBUF eviction.

Matmuls run in float16 (PE streams 16-bit operands at 1 row/cycle at
2.4 GHz vs fp32r's effective half rate) with fp32 PSUM accumulation;
x/w are rounded to fp16 on the host and the output is stored fp16
(rel err ~3e-4, under the 2e-3 gate). Inputs are pre-tiled on the host
into the exact SBUF layouts so every DMA line is a contiguous >=2KB
per-partition run:
  x  -> [NB, 128, KO, 512]   (nb b-tile, ki partition, ko k-subtile, b col)
  w  -> [128, MO, KO, 128]   (ki partition, mo o-tile, ko k-subtile, m col)
  out <- [NB, 128, MO, 512]  (nb, o-partition, mo o-tile, b col)

Schedule (traced): ~7us fixed runtime preamble precedes the first
user DMA; HWDGE rings stream queued transfers back-to-back (receipt
only delays semaphore visibility by ~2us). So: w rides the ACT ring
(singles first, then pairs — PE eats one 256KB w tile per 1.7us), x
rides the SP ring (nb0 in k-halves for an early PE start, then whole
1MB nb stripes), outputs ride SWDGE + whichever HWDGE ring is idle.
While the first chunks are in flight the tensor engine runs ~44 dummy
128-wide matmuls on a zeroed tile so the PE_HAM clock gate is already
at 8/8 (2.4 GHz) when real data lands — otherwise the first ~3.4us of
matmuls run at 1.2 GHz.
"""

import numpy as np

import concourse.bass as bass  # noqa: F401
import concourse.mybir as mybir
import concourse.tile as tile
from concourse import bacc
from concourse.bass_utils import run_bass_kernel_spmd

B, IN, OUT = 16384, 1024, 1024
N_CORES = 8
BS = B // N_CORES  # 2048 batch rows per core
P = 128  # SBUF partitions
KO = IN // P  # 8 k-subtiles of the contraction dim
MO = OUT // P  # 8 out-feature tiles (psum partition dim)
NB_TILE = 512  # matmul free dim = one fp32 PSUM bank
NB = BS // NB_TILE  # 4 batch tiles per core
N_WARM = 60  # dummy matmuls to hold PE_HAM at 8/8 until real data lands

_CACHE = {}


def _build_nc():
    f32 = mybir.dt.float32
    f16 = mybir.dt.float16

    nc = bacc.Bacc("TRN2", target_bir_lowering=False)
    x_d = nc.dram_tensor("x_t", [NB, P, KO, NB_TILE], f16, kind="ExternalInput")
    w_d = nc.dram_tensor("w_t", [P, MO, KO, P], f16, kind="ExternalInput")
    b_d = nc.dram_tensor("bias_t", [P, MO], f32, kind="ExternalInput")
    o_d = nc.dram_tensor("out_t", [NB, P, MO, NB_TILE], f16, kind="ExternalOutput")

    with tile.TileContext(nc) as tc:
        with (
            tc.tile_pool(name="wp", bufs=1) as wp,
            tc.tile_pool(name="xp", bufs=1) as xp,
            tc.tile_pool(name="cp", bufs=1) as cp,
            tc.tile_pool(name="op", bufs=1) as op,
            tc.tile_pool(name="ps", bufs=4, space="PSUM") as ps,
            tc.tile_pool(name="pw", bufs=1, space="PSUM") as pw,
        ):
            # PE warmup: zero tile -> dummy matmuls keep the PE busy (and
            # the HAM clock-gate warming) while the first DMAs are in
            # flight. Results land in a scratch PSUM bank, never read.
            wz = cp.tile([P, P], f16)
            nc.vector.memset(wz[:], 0.0)
            psz = pw.tile([P, P], f32)
            for _ in range(N_WARM):
                nc.tensor.matmul(psz[:], wz[:], wz[:], start=True, stop=True)

            # bias rides SWDGE (idle until outputs start)
            bias_sb = cp.tile([P, MO], f32)
            nc.gpsimd.dma_start(bias_sb[:], b_d[:])

            w_sb = [None] * MO
            x_parts = [[] for _ in range(NB)]

            def load_w(mos, engine):
                t = wp.tile([P, len(mos), KO, P], f16, tag=f"w{mos[0]}")
                engine.dma_start(t[:], w_d[:, mos[0] : mos[0] + len(mos)])
                for i, mo in enumerate(mos):
                    w_sb[mo] = t[:, i]

            def load_x(nb, k0, kn, engine):
                t = xp.tile([P, kn, NB_TILE], f16, tag=f"x{nb}_{k0}")
                engine.dma_start(t[:], x_d[nb, :, k0 : k0 + kn])
                x_parts[nb].append((k0, kn, t))

            # ALL inputs ride one ring (SP HWDGE) in exact PE consumption
            # order. Two concurrent input queues share the 16 SDMA engines
            # round-robin at *packet* granularity, so the queue with bigger
            # per-partition runs starves the other (traced: w got 115GB/s
            # vs x 218GB/s -> 3.5us mid-kernel stalls). One queue at ~350GB/s
            # delivers every tile ahead of its first use instead.
            KH = KO // 2
            load_w([0], nc.sync)
            load_x(0, 0, KH, nc.sync)
            load_x(0, KH, KH, nc.sync)
            load_w([1], nc.sync)
            load_w([2], nc.sync)
            load_w([3], nc.sync)
            load_w([4], nc.sync)
            load_x(1, 0, KO, nc.sync)
            load_w([5], nc.sync)
            load_w([6], nc.sync)
            load_w([7], nc.sync)
            load_x(2, 0, KO, nc.sync)
            load_x(3, 0, KO, nc.sync)

            def x_slice(nb, k):
                for k0, kn, t in x_parts[nb]:
                    if k0 <= k < k0 + kn:
                        return t[:, k - k0]
                raise AssertionError

            MH = MO // 2  # output DMA chunk = half an nb stripe (512KB)
            out_engines = {
                (0, 0): nc.gpsimd,
                (0, 1): nc.scalar,
                (1, 0): nc.gpsimd,
                (1, 1): nc.scalar,
                (2, 0): nc.gpsimd,
                (2, 1): nc.scalar,
            }
            # last nb ships per-mo (128KB chunks) so the tail after the
            # final matmul is one small transfer; the final chunk rides
            # the SP ring (fastest first-byte, input work long done).
            nb3_eng = [nc.gpsimd, nc.scalar, nc.gpsimd, nc.scalar,
                       nc.gpsimd, nc.scalar, nc.gpsimd, nc.sync]

            for nb in range(NB):
                if nb < NB - 1:
                    ot = [op.tile([P, MH, NB_TILE], f16, tag=f"o{nb}_{h}",
                                  name=f"o{nb}_{h}")
                          for h in range(2)]
                else:
                    ot = [op.tile([P, NB_TILE], f16, tag=f"o{nb}_{q}",
                                  name=f"o{nb}_{q}")
                          for q in range(MO)]
                NH2 = NB_TILE // 2
                for mo in range(MO):
                    pt = ps.tile([P, NB_TILE], mybir.dt.float32)
                    for h in range(2):
                        for k in range(KO):
                            nc.tensor.matmul(
                                pt[:, h * NH2 : (h + 1) * NH2],
                                w_sb[mo][:, k],
                                x_slice(nb, k)[:, h * NH2 : (h + 1) * NH2],
                                start=(k == 0),
                                stop=(k == KO - 1),
                            )
                    if nb < NB - 1:
                        h, i = divmod(mo, MH)
                        dst = ot[h][:, i]
                        nc.vector.tensor_scalar_add(
                            dst, pt[:], bias_sb[:, mo : mo + 1]
                        )
                        if mo == MH - 1:
                            out_engines[(nb, 0)].dma_start(
                                o_d[nb, :, 0:MH], ot[0][:]
                            )
                        elif mo == MO - 1:
                            out_engines[(nb, 1)].dma_start(
                                o_d[nb, :, MH:MO], ot[1][:]
                            )
                    elif mo < MO - 1:
                        nc.vector.tensor_scalar_add(
                            ot[mo][:], pt[:], bias_sb[:, mo : mo + 1]
                        )
                        nb3_eng[mo].dma_start(o_d[nb, :, mo], ot[mo][:])
                    else:
                        # final tile: evict in halves on two engines in
                        # parallel, each issuing its own half-DMA, so the
                        # tail after the last matmul is minimal.
                        NH = NB_TILE // 2
                        nc.vector.tensor_scalar_add(
                            ot[mo][:, :NH], pt[:, :NH], bias_sb[:, mo : mo + 1]
                        )
                        nc.sync.dma_start(
                            o_d[nb, :, mo, :NH], ot[mo][:, :NH]
                        )
                        nc.vector.tensor_scalar_add(
                            ot[mo][:, NH:], pt[:, NH:], bias_sb[:, mo : mo + 1]
                        )
                        nc.scalar.dma_start(
                            o_d[nb, :, mo, NH:], ot[mo][:, NH:]
                        )

    nc.finalize()
    return nc


def _get_nc():
    if "nc" not in _CACHE:
        _CACHE["nc"] = _build_nc()
    return _CACHE["nc"]


def _prep_inputs(x, base_w, base_b, spline_w):
    x = np.asarray(x, dtype=np.float32)
    base_w = np.asarray(base_w, dtype=np.float32)
    base_b = np.asarray(base_b, dtype=np.float32)
    spline_w = np.asarray(spline_w, dtype=np.float32)

    s_feats = spline_w.shape[1]
    spline_input = np.linspace(0.0, 1.0, s_feats, dtype=np.float32)
    bias = (base_b + spline_w @ spline_input).astype(np.float32)  # [OUT]

    # w_dev[ki, mo, ko, m] = base_w[mo*P + m, ko*P + ki]
    w_dev = np.ascontiguousarray(
        base_w.astype(np.float16).reshape(MO, P, KO, P).transpose(3, 0, 2, 1)
    )
    # bias_dev[p, mo] = bias[mo*P + p]
    bias_dev = np.ascontiguousarray(bias.reshape(MO, P).T)

    x16 = x.astype(np.float16)
    in_maps = []
    for c in range(N_CORES):
        xs = x16[c * BS : (c + 1) * BS]  # [BS, IN]
        # x_dev[nb, ki, ko, col] = xs[nb*NB_TILE + col, ko*P + ki]
        x_dev = np.ascontiguousarray(
            xs.reshape(NB, NB_TILE, KO, P).transpose(0, 3, 2, 1)
        )
        in_maps.append({"x_t": x_dev, "w_t": w_dev, "bias_t": bias_dev})
    return in_maps


def _run(inputs, trace=False, tmpdir=None):
    nc = _get_nc()
    in_maps = _prep_inputs(**inputs)
    res = run_bass_kernel_spmd(
        nc, in_maps, core_ids=list(range(N_CORES)), trace=trace, tmpdir=tmpdir
    )
    outs = []
    for c in range(N_CORES):
        arr = np.asarray(res.results[c]["out_t"])  # [NB, P, MO, NB_TILE] f16
        # out_core[nb*NB_TILE + col, mo*P + p] = arr[nb, p, mo, col]
        outs.append(arr.transpose(0, 3, 2, 1).reshape(BS, OUT))
    full = np.concatenate(outs, axis=0).astype(np.float32)
    return np.ascontiguousarray(full), res


def kernel(**inputs) -> np.ndarray:
    out, _ = _run(inputs, trace=False)
    return out


# revision 16
# speedup vs baseline: 1.1814x; 1.0111x over previous
"""KANLinear forward on 8 TRN2 NeuronCores.

Reference computes
    out = x @ base_w.T + base_b + spline_w @ linspace(0, 1, S)
The spline branch is batch-independent, so it folds into a single bias
vector on the host. The device kernel is a data-parallel matmul: each
core computes a [2048, 1024] batch shard as out.T tiles ([out-feature
partitions, batch free dim]) so the per-feature bias is a per-partition
scalar add fused into the PSUM->SBUF eviction.

Matmuls run in float16 (PE streams 16-bit operands at 1 row/cycle at
2.4 GHz vs fp32r's effective half rate) with fp32 PSUM accumulation;
x/w are rounded to fp16 on the host and the output is stored fp16
(rel err ~3e-4, under the 2e-3 gate). Inputs are pre-tiled on the host
into the exact SBUF layouts so every DMA line is a contiguous >=2KB
per-partition run:
  x  -> [NB, 128, KO, 512]   (nb b-tile, ki partition, ko k-subtile, b col)
  w  -> [128, MO, KO, 128]   (ki partition, mo o-tile, ko k-subtile, m col)
  out <- [NB, 128, MO, 512]  (nb, o-partition, mo o-tile, b col)

Schedule (traced): ~7us fixed runtime preamble precedes the first
user DMA; HWDGE rings stream queued transfers back-to-back (receipt
only delays semaphore visibility by ~2us). So: w rides the ACT ring
(singles first, then pairs — PE eats one 256KB w tile per 1.7us), x
rides the SP ring (nb0 in k-halves for an early PE start, then whole
1MB nb stripes), outputs ride SWDGE + whichever HWDGE ring is idle.
While the first chunks are in flight the tensor engine runs ~44 dummy
128-wide matmuls on a zeroed tile so the PE_HAM clock gate is already
at 8/8 (2.4 GHz) when real data lands — otherwise the first ~3.4us of
matmuls run at 1.2 GHz.
"""

import numpy as np

import concourse.bass as bass  # noqa: F401
import concourse.mybir as mybir
import concourse.tile as tile
from concourse import bacc
from concourse.bass_utils import run_bass_kernel_spmd

B, IN, OUT = 16384, 1024, 1024
N_CORES = 8
BS = B // N_CORES  # 2048 batch rows per core
P = 128  # SBUF partitions
KO = IN // P  # 8 k-subtiles of the contraction dim
MO = OUT // P  # 8 out-feature tiles (psum partition dim)
NB_TILE = 512  # matmul free dim = one fp32 PSUM bank
NB = BS // NB_TILE  # 4 batch tiles per core
N_WARM = 60  # dummy matmuls to hold PE_HAM at 8/8 until real data lands

_CACHE = {}


def _build_nc():
    f32 = mybir.dt.float32
    f16 = mybir.dt.float16

    nc = bacc.Bacc("TRN2", target_bir_lowering=False)
    x_d = nc.dram_tensor("x_t", [NB, P, KO, NB_TILE], f16, kind="ExternalInput")
    w_d = nc.dram_tensor("w_t", [P, MO, KO, P], f16, kind="ExternalInput")
    b_d = nc.dram_tensor("bias_t", [P, MO], f32, kind="ExternalInput")
    o_d = nc.dram_tensor("out_t", [NB, P, MO, NB_TILE], f16, kind="ExternalOutput")

    with tile.TileContext(nc) as tc:
        with (
            tc.tile_pool(name="wp", bufs=1) as wp,
            tc.tile_pool(name="xp", bufs=1) as xp,
            tc.tile_pool(name="cp", bufs=1) as cp,
            tc.tile_pool(name="op", bufs=1) as op,
            tc.tile_pool(name="ps", bufs=4, space="PSUM") as ps,
            tc.tile_pool(name="ps3", bufs=1, space="PSUM") as ps3,
            tc.tile_pool(name="pw", bufs=1, space="PSUM") as pw,
        ):
            # PE warmup: zero tile -> dummy matmuls keep the PE busy (and
            # the HAM clock-gate warming) while the first DMAs are in
            # flight. Results land in a scratch PSUM bank, never read.
            wz = cp.tile([P, P], f16)
            nc.vector.memset(wz[:], 0.0)
            psz = pw.tile([P, P], f32)
            for _ in range(N_WARM):
                nc.tensor.matmul(psz[:], wz[:], wz[:], start=True, stop=True)

            # bias rides SWDGE (idle until outputs start)
            bias_sb = cp.tile([P, MO], f32)
            nc.gpsimd.dma_start(bias_sb[:], b_d[:])

            w_sb = [None] * MO
            x_parts = [[] for _ in range(NB)]

            def load_w(mos, engine):
                t = wp.tile([P, len(mos), KO, P], f16, tag=f"w{mos[0]}")
                engine.dma_start(t[:], w_d[:, mos[0] : mos[0] + len(mos)])
                for i, mo in enumerate(mos):
                    w_sb[mo] = t[:, i]

            def load_x(nb, k0, kn, engine):
                t = xp.tile([P, kn, NB_TILE], f16, tag=f"x{nb}_{k0}")
                engine.dma_start(t[:], x_d[nb, :, k0 : k0 + kn])
                x_parts[nb].append((k0, kn, t))

            # ALL inputs ride one ring (SP HWDGE) in exact PE consumption
            # order. Two concurrent input queues share the 16 SDMA engines
            # round-robin at *packet* granularity, so the queue with bigger
            # per-partition runs starves the other (traced: w got 115GB/s
            # vs x 218GB/s -> 3.5us mid-kernel stalls). One queue at ~350GB/s
            # delivers every tile ahead of its first use instead.
            KH = KO // 2
            load_w([0], nc.sync)
            load_x(0, 0, KH, nc.sync)
            load_x(0, KH, KH, nc.sync)
            load_w([1], nc.sync)
            load_w([2], nc.sync)
            load_w([3], nc.sync)
            load_w([4], nc.sync)
            load_x(1, 0, KO, nc.sync)
            load_w([5], nc.sync)
            load_w([6], nc.sync)
            load_w([7], nc.sync)
            load_x(2, 0, KO, nc.sync)
            load_x(3, 0, KO, nc.sync)

            def x_slice(nb, k):
                for k0, kn, t in x_parts[nb]:
                    if k0 <= k < k0 + kn:
                        return t[:, k - k0]
                raise AssertionError

            MH = MO // 2  # output DMA chunk = half an nb stripe (512KB)
            out_engines = {
                (0, 0): nc.gpsimd,
                (0, 1): nc.scalar,
                (1, 0): nc.gpsimd,
                (1, 1): nc.scalar,
                (2, 0): nc.gpsimd,
                (2, 1): nc.scalar,
            }
            # last nb ships per-mo (128KB chunks) so the tail after the
            # final matmul is one small transfer; the final chunk rides
            # the SP ring (fastest first-byte, input work long done).
            nb3_eng = [nc.gpsimd, nc.scalar, nc.gpsimd, nc.scalar,
                       nc.gpsimd, nc.scalar, nc.gpsimd, nc.sync]

            for nb in range(NB):
                if nb < NB - 1:
                    ot = [op.tile([P, MH, NB_TILE], f16, tag=f"o{nb}_{h}",
                                  name=f"o{nb}_{h}")
                          for h in range(2)]
                else:
                    ot = [op.tile([P, NB_TILE], f16, tag=f"o{nb}_{q}",
                                  name=f"o{nb}_{q}")
                          for q in range(MO)]
                NH2 = NB_TILE // 2
                for mo in range(MO):
                    if nb == NB - 1 and mo == MO - 1:
                        # final tile: two independent half-psum groups so
                        # the first half's eviction + DMA overlap the
                        # second half's matmuls — the tail after the last
                        # matmul is one small transfer per ring.
                        for h, eng in ((0, nc.sync), (1, nc.scalar)):
                            pt = ps3.tile([P, NH2], mybir.dt.float32,
                                          tag=f"fin{h}")
                            for k in range(KO):
                                nc.tensor.matmul(
                                    pt[:],
                                    w_sb[mo][:, k],
                                    x_slice(nb, k)[:, h * NH2 : (h + 1) * NH2],
                                    start=(k == 0),
                                    stop=(k == KO - 1),
                                )
                            sl = slice(h * NH2, (h + 1) * NH2)
                            nc.vector.tensor_scalar_add(
                                ot[mo][:, sl], pt[:], bias_sb[:, mo : mo + 1]
                            )
                            eng.dma_start(o_d[nb, :, mo, sl], ot[mo][:, sl])
                        continue
                    if nb == 0 and mo == 0:
                        # first tile: k-outer over two independent
                        # half-bank psum groups (start clears a whole
                        # bank, so interleaved halves must not share one)
                        # — PE consumes the first k-half chunk at half
                        # pace while the second is still in flight.
                        hp = [ps3.tile([P, NH2], mybir.dt.float32,
                                       tag=f"fin{h}", name=f"hp{h}")
                              for h in range(2)]
                        for k in range(KO):
                            for h in range(2):
                                nc.tensor.matmul(
                                    hp[h][:],
                                    w_sb[mo][:, k],
                                    x_slice(nb, k)[:, h * NH2 : (h + 1) * NH2],
                                    start=(k == 0),
                                    stop=(k == KO - 1),
                                )
                        for h in range(2):
                            nc.vector.tensor_scalar_add(
                                ot[0][:, 0, h * NH2 : (h + 1) * NH2],
                                hp[h][:],
                                bias_sb[:, mo : mo + 1],
                            )
                        continue
                    pt = ps.tile([P, NB_TILE], mybir.dt.float32)
                    for h in range(2):
                        for k in range(KO):
                            nc.tensor.matmul(
                                pt[:, h * NH2 : (h + 1) * NH2],
                                w_sb[mo][:, k],
                                x_slice(nb, k)[:, h * NH2 : (h + 1) * NH2],
                                start=(k == 0),
                                stop=(k == KO - 1),
                            )
                    if nb < NB - 1:
                        h, i = divmod(mo, MH)
                        dst = ot[h][:, i]
                        nc.vector.tensor_scalar_add(
                            dst, pt[:], bias_sb[:, mo : mo + 1]
                        )
                        if mo == MH - 1:
                            out_engines[(nb, 0)].dma_start(
                                o_d[nb, :, 0:MH], ot[0][:]
                            )
                        elif mo == MO - 1:
                            out_engines[(nb, 1)].dma_start(
                                o_d[nb, :, MH:MO], ot[1][:]
                            )
                    else:
                        nc.vector.tensor_scalar_add(
                            ot[mo][:], pt[:], bias_sb[:, mo : mo + 1]
                        )
                        nb3_eng[mo].dma_start(o_d[nb, :, mo], ot[mo][:])

    nc.finalize()
    return nc


def _get_nc():
    if "nc" not in _CACHE:
        _CACHE["nc"] = _build_nc()
    return _CACHE["nc"]


def _prep_inputs(x, base_w, base_b, spline_w):
    x = np.asarray(x, dtype=np.float32)
    base_w = np.asarray(base_w, dtype=np.float32)
    base_b = np.asarray(base_b, dtype=np.float32)
    spline_w = np.asarray(spline_w, dtype=np.float32)

    s_feats = spline_w.shape[1]
    spline_input = np.linspace(0.0, 1.0, s_feats, dtype=np.float32)
    bias = (base_b + spline_w @ spline_input).astype(np.float32)  # [OUT]

    # w_dev[ki, mo, ko, m] = base_w[mo*P + m, ko*P + ki]
    w_dev = np.ascontiguousarray(
        base_w.astype(np.float16).reshape(MO, P, KO, P).transpose(3, 0, 2, 1)
    )
    # bias_dev[p, mo] = bias[mo*P + p]
    bias_dev = np.ascontiguousarray(bias.reshape(MO, P).T)

    x16 = x.astype(np.float16)
    in_maps = []
    for c in range(N_CORES):
        xs = x16[c * BS : (c + 1) * BS]  # [BS, IN]
        # x_dev[nb, ki, ko, col] = xs[nb*NB_TILE + col, ko*P + ki]
        x_dev = np.ascontiguousarray(
            xs.reshape(NB, NB_TILE, KO, P).transpose(0, 3, 2, 1)
        )
        in_maps.append({"x_t": x_dev, "w_t": w_dev, "bias_t": bias_dev})
    return in_maps


def _run(inputs, trace=False, tmpdir=None):
    nc = _get_nc()
    in_maps = _prep_inputs(**inputs)
    res = run_bass_kernel_spmd(
        nc, in_maps, core_ids=list(range(N_CORES)), trace=trace, tmpdir=tmpdir
    )
    outs = []
    for c in range(N_CORES):
        arr = np.asarray(res.results[c]["out_t"])  # [NB, P, MO, NB_TILE] f16
        # out_core[nb*NB_TILE + col, mo*P + p] = arr[nb, p, mo, col]
        outs.append(arr.transpose(0, 3, 2, 1).reshape(BS, OUT))
    full = np.concatenate(outs, axis=0).astype(np.float32)
    return np.ascontiguousarray(full), res


def kernel(**inputs) -> np.ndarray:
    out, _ = _run(inputs, trace=False)
    return out
